# revision 37
# baseline (speedup 1.0000x reference)
"""Trainium2 Bass kernel for nn_MultiHeadMLP (multi-head attention over a fixed
memory bank of 2048 slots/head, with L2-normalized queries/keys).

Sharding: data-parallel over the 4096-token sequence across 8 NeuronCores
(512 rows each); keys/values/projections replicated. No collectives.

Weight preprocessing on host (normalize+scale keys incl. attn_scale sign,
transposes, dtype casts); the device does all x-dependent compute.

v2: two structural changes over the 124us baseline.

1. fp8e4 DoubleRow attention-score matmuls. The attention contraction is
   only d=128, so DoubleRow (which wants 2x128 paired rows) is fed a
   second, zeroed pair-plane: qhat lives as [128, H, 3, SC] f8 with
   planes 0/2 zero and plane 1 the data; matmul t pairs kts tiles
   (t, t+1) with qhat planes (1, 2) = (data, zero), and the last tile
   t=15 pairs kts (14, 15) with planes (0, 1) = (zero, data) so no kts
   padding is needed. Halves the PE cost of the biggest matmul; a numpy
   error study shows fp8 q/k moves rel-err only 0.0067 -> 0.0084
   (softmax renormalization crushes pre-softmax quantization noise;
   fp8 on E/v/Wo would be 2.6-3.9e-2 and those stay bf16).

2. Software-pipelined rep loop. The trace showed a 27us ACT idle gap per
   rep: the output projection C ran as a serial PE block at rep end, then
   the next rep's query pipeline had to restart before the first exp.
   Now each body() iteration runs: B(i) (attention, ACT-paced by exp)
   with C(i-1)'s matmuls injected 2-per-group into B's PE slack (using
   the qt/ssq PSUM banks in their idle g4..g7 windows), loads for rep
   i+1 issued up front, and A(h0,h1) of rep i+1 emitted at body end.
   Prologue runs loads(0)+A01(0); epilogue runs the final C. For the
   first body, C(-1) reads a zeroed ynorm buffer and its stores are
   overwritten by the epilogue on the same DMA queue.

Engine budget per rep per core (cost-model):
  ACT ~77.5us busy = exp(att) 64x[128,2,512] + rstd Ln/Exp  <- pacer
  PE  ~66us busy = proj-DR + attn-DR + y-bf16 + ssq + den + Wo-bf16
  DVE ~50us: sq, qhat-f8 cast-mult, pairwise E adds, ynorm divide, out
  staging; Pool: lvl-0 E adds + store DMA issue.
DMA queues: x/Wq/Wo on sync, kts/v on the scalar queue, stores on gpsimd.

Single-activation-table design: ACT uses only Ln / Exp (one shared table;
rstd = exp(-0.5*ln(ssq+eps))) -> zero table reloads.
"""
import numpy as np
import ml_dtypes

import concourse.bacc as bacc
import concourse.mybir as mybir
import concourse.tile as tile
from concourse.bass_utils import run_bass_kernel_spmd

# The stock activation-table assigner first-fits Exp into 'exp_and_others',
# which lacks Ln, so a kernel using Ln+Exp ping-pongs tables (1.3us per
# reload). Hide those functions from every other table (keeping table
# order/IDs intact -- act_func_set_id is positional) so all of this
# kernel's activations land on 'natural_log_exp_and_others': zero reloads.
import concourse.hw_specs as _hw_specs
_ORIG_GAT = _hw_specs.get_activation_tables
_OUR_FNS = None  # set below after AF is defined


def _patched_gat(arch):
    tabs = _ORIG_GAT(arch)
    keep = "natural_log_exp_and_others"
    if keep not in tabs or not _OUR_FNS <= tabs[keep]:
        return tabs
    return {name: (fns if name == keep else fns - _OUR_FNS)
            for name, fns in tabs.items()}


_hw_specs.get_activation_tables = _patched_gat
bacc.get_activation_tables = _patched_gat

B, S, D = 1, 4096, 1024
H, HD, K = 8, 128, 2048
EPS = 1e-6
N_CORES = 8
SC = S // N_CORES      # 512 sequence rows per core
KT = K // 128          # 16 key tiles per head
MT = D // 128          # 8 contraction tiles for D
NG = KT // 2           # 8 attention groups per head
f32 = mybir.dt.float32
bf16 = mybir.dt.bfloat16
f8 = mybir.dt.float8e4
AF = mybir.ActivationFunctionType
OP = mybir.AluOpType
DR = mybir.MatmulPerfMode.DoubleRow
NP_F8 = ml_dtypes.float8_e4m3
NP_BF16 = ml_dtypes.bfloat16
_OUR_FNS = {AF.Exp, AF.Square, AF.Ln}


def build_nc(reps=1):
    nc = bacc.Bacc("TRN2", target_bir_lowering=False, debug=False, num_devices=N_CORES)
    xT = nc.dram_tensor("xT", [128, MT, SC], f8, kind="ExternalInput").ap()
    Wq = nc.dram_tensor("Wq", [128, MT, D], f8, kind="ExternalInput").ap()
    kts = nc.dram_tensor("kts", [128, H, KT, 128], f8, kind="ExternalInput").ap()
    v = nc.dram_tensor("v", [128, H, KT, HD], bf16, kind="ExternalInput").ap()
    Wo = nc.dram_tensor("Wo", [128, H, D], bf16, kind="ExternalInput").ap()
    out = nc.dram_tensor("out", [SC, D], f32, kind="ExternalOutput").ap()

    with tile.TileContext(nc) as tc:
        with tc.tile_pool(name="consts", bufs=1) as consts, \
             tc.tile_pool(name="wq01_p", bufs=2) as wq01_p, \
             tc.tile_pool(name="wq27_p", bufs=1) as wq27_p, \
             tc.tile_pool(name="xt_p", bufs=2) as xt_p, \
             tc.tile_pool(name="kts_p", bufs=2) as kts_p, \
             tc.tile_pool(name="v_p", bufs=2) as v_p, \
             tc.tile_pool(name="wo_p", bufs=2) as wo_p, \
             tc.tile_pool(name="ynorm_p", bufs=2) as ynorm_p, \
             tc.tile_pool(name="ps_att", bufs=2, space="PSUM") as ps_att, \
             tc.tile_pool(name="ps_y", bufs=1, space="PSUM") as ps_y, \
             tc.tile_pool(name="ps_qt", bufs=1, space="PSUM") as ps_qt, \
             tc.tile_pool(name="ps_sq", bufs=1, space="PSUM") as ps_sq, \
             tc.tile_pool(name="ps_den", bufs=1, space="PSUM") as ps_den, \
             tc.tile_pool(name="sqtmp", bufs=2) as sqtmp, \
             tc.tile_pool(name="qtb_p", bufs=2) as qtb_p, \
             tc.tile_pool(name="rec_p", bufs=2) as rec_p, \
             tc.tile_pool(name="exp_p", bufs=4) as exp_p, \
             tc.tile_pool(name="acc_p", bufs=5) as acc_p, \
             tc.tile_pool(name="outsb", bufs=1) as outsb:

            # constants: initialized once, shared by every rep
            eps_t = consts.tile([128, 1], f32)
            nc.vector.memset(eps_t[:], EPS)
            ones_f = consts.tile([128, 128], f32)
            nc.vector.memset(ones_f[:], 1.0)
            ones_b = consts.tile([128, 128], bf16)
            nc.vector.tensor_copy(out=ones_b[:], in_=ones_f[:])
            # qhat with interleaved zero planes for the DoubleRow attention
            # trick: plane 2h+1 holds head h's data, even planes stay zero
            # forever (plane 16 is the trailing zero for h=7).
            qhat_sb = consts.tile([128, 2 * H + 1, SC], f8)
            for zp in range(0, 2 * H + 1, 2):
                nc.vector.memset(qhat_sb[:, zp, :], 0.0)

            st = {}   # cross-emission ring state

            def do_loads(cold=False):
                """Issue the full set of input loads; returns tile handles.

                wq is split: heads 0/1 (used by the next-rep A01 at body
                end) are double-buffered; heads 2..7 single-buffered and
                loaded last on the sync queue so their buffer-free wait
                (last A7 read, ~60% into the previous body) blocks nothing
                else."""
                xt_sb = xt_p.tile([128, MT, SC], f8, name="xt_sb")
                wq01_sb = wq01_p.tile([128, MT, 256], f8, name="wq01_sb")
                wq27_sb = wq27_p.tile([128, MT, 768], f8, name="wq27_sb")
                kts_sb = kts_p.tile([128, H, KT, 128], f8, name="kts_sb")
                v_sb = v_p.tile([128, H, KT, HD], bf16, name="v_sb")
                wo_sb = wo_p.tile([128, H, D], bf16, name="wo_sb")
                # All loads on the SP queue (it has no compute to stall;
                # a trigger waiting on buffer-free blocks nothing).
                # cold=True (iteration prologue, after the For_i barrier):
                # strictly earliest-needed-first so the A01 chain and the
                # first attention groups start ASAP. Steady bodies load a
                # full rep ahead, so order barely matters there.
                if cold:
                    nc.sync.dma_start(out=wq01_sb[:], in_=Wq[:, :, 0:256])
                    for t in range(MT // 2):
                        nc.sync.dma_start(out=xt_sb[:, 2 * t:2 * t + 2, :],
                                          in_=xT[:, 2 * t:2 * t + 2, :])
                    nc.sync.dma_start(out=kts_sb[:, 0, :, :],
                                      in_=kts[:, 0, :, :])
                    nc.sync.dma_start(out=v_sb[:, 0, :, :], in_=v[:, 0, :, :])
                    for hp in range(3):
                        nc.sync.dma_start(
                            out=wq27_sb[:, :, hp * 256:(hp + 1) * 256],
                            in_=Wq[:, :, 256 + hp * 256:256 + (hp + 1) * 256])
                    for h in range(1, H):
                        nc.sync.dma_start(out=kts_sb[:, h, :, :],
                                          in_=kts[:, h, :, :])
                        nc.sync.dma_start(out=v_sb[:, h, :, :],
                                          in_=v[:, h, :, :])
                    nc.sync.dma_start(out=wo_sb[:], in_=Wo)
                else:
                    for h in range(H):
                        nc.sync.dma_start(out=kts_sb[:, h, :, :],
                                          in_=kts[:, h, :, :])
                        nc.sync.dma_start(out=v_sb[:, h, :, :],
                                          in_=v[:, h, :, :])
                    nc.sync.dma_start(out=wq01_sb[:], in_=Wq[:, :, 0:256])
                    for t in range(MT // 2):
                        nc.sync.dma_start(out=xt_sb[:, 2 * t:2 * t + 2, :],
                                          in_=xT[:, 2 * t:2 * t + 2, :])
                    nc.sync.dma_start(out=wo_sb[:], in_=Wo)
                return {"xt": xt_sb, "wq01": wq01_sb, "wq27": wq27_sb,
                        "kts": kts_sb, "v": v_sb, "wo": wo_sb}

            def do_load_wq27(wq27_sb):
                # single-buffered: must be EMITTED after this body's last
                # A-rest read of the previous epoch, else the tile
                # framework's same-buffer epoch ordering deadlocks the
                # hardware loop (load waits readers, readers wait load).
                for hp in range(3):
                    nc.sync.dma_start(
                        out=wq27_sb[:, :, hp * 256:(hp + 1) * 256],
                        in_=Wq[:, :, 256 + hp * 256:256 + (hp + 1) * 256])

            def emit_a(h, xt_sb, loads, qt_pool=None, ssq_pool=None):
                # query projection + normalization for head h -> qhat
                # data plane 2h+1. The A01 pair at body end passes split
                # pools so the two heads' chains overlap instead of
                # serializing through one PSUM bank.
                if h < 2:
                    wq_sb, col = loads["wq01"], h * 128
                else:
                    wq_sb, col = loads["wq27"], (h - 2) * 128
                qp, qtag = (qt_pool, "qt") if qt_pool is not None else (ps_qt, "qt")
                if qt_pool is ps_sq:
                    qtag = "ssq"
                sp, stag = (ssq_pool, "ssq") if ssq_pool is not None else (ps_sq, "ssq")
                if ssq_pool is ps_den:
                    stag = "den"
                qt_ps = qp.tile([128, SC], f32, tag=qtag, name="qt")
                for t in range(MT // 2):
                    nc.tensor.matmul(
                        qt_ps[:],
                        wq_sb[:, 2 * t:2 * t + 2, col:col + 128],
                        xt_sb[:, 2 * t:2 * t + 2, :],
                        start=(t == 0), stop=(t == MT // 2 - 1),
                        perf_mode=DR)
                # DVE cannot read two PSUM operands in one instruction, so
                # stage qt into SBUF bf16 first (also frees the qt PSUM
                # bank early and lets the square run in the fast 2x mode).
                qtb = qtb_p.tile([128, SC], bf16, tag="qtb", name="qtb")
                nc.vector.tensor_copy(out=qtb[:], in_=qt_ps[:])
                sq = sqtmp.tile([128, SC], bf16, tag="sq", name="sq")
                nc.vector.tensor_tensor(out=sq[:], in0=qtb[:],
                                        in1=qtb[:], op=OP.mult)
                ssq_ps = sp.tile([128, SC], f32, tag=stag, name="ssq")
                nc.tensor.matmul(ssq_ps[:], ones_b[:], sq[:],
                                 start=True, stop=True)
                lns = sqtmp.tile([128, SC], f32, tag="lns", name="lns")
                nc.scalar.activation(out=lns[:], in_=ssq_ps[:],
                                     func=AF.Ln, bias=eps_t[:], scale=1.0)
                rstd = sqtmp.tile([128, SC], f32, tag="rstd", name="rstd")
                nc.scalar.activation(out=rstd[:], in_=lns[:],
                                     func=AF.Exp, bias=0.0, scale=-0.5)
                nc.vector.tensor_tensor(out=qhat_sb[:, 2 * h + 1, :],
                                        in0=qtb[:], in1=rstd[:], op=OP.mult)

            def c_chunk_mm(j, hh, yn, wo_sb):
                # output-projection accumulation step hh of chunk j
                si, oc = j // 2, j % 2
                pool, tg = (ps_qt, "qt") if j % 2 == 0 else (ps_sq, "ssq")
                if hh == 0:
                    st[("ops", j)] = pool.tile([128, SC], f32, tag=tg,
                                               name="ops")
                o_ps = st[("ops", j)]
                nc.tensor.matmul(
                    o_ps[:],
                    yn[:, hh, si * 128:(si + 1) * 128],
                    wo_sb[:, hh, oc * 512:(oc + 1) * 512],
                    start=(hh == 0), stop=(hh == H - 1))
                if hh == H - 1:
                    o_sb = outsb.tile([128, 512], f32, tag="osb", name="osb")
                    nc.vector.tensor_copy(out=o_sb[:], in_=o_ps[:])
                    nc.gpsimd.dma_start(
                        out=out[si * 128:(si + 1) * 128,
                                oc * 512:(oc + 1) * 512],
                        in_=o_sb[:])

            def body(first=False, last=False):
                cur = st["loads"]
                nxt = None if last else do_loads()
                ynorm = ynorm_p.tile([128, H, SC], bf16, name="ynorm")
                # C chunks read the CURRENT wo buffer (same values as the
                # previous rep's -- Wo reloads identically every rep) so
                # the wo load pipeline never waits on late chunk readers.
                yprev, woprev = st.get("ynorm_prev"), cur["wo"]

                yt_of = {}
                pacc_of = {h: [] for h in range(H)}
                pend = []   # two-step lag queue of (h, g, exp_sb)

                def consume(h, g, exp_sb):
                    for i in range(2):
                        t = 2 * g + i
                        nc.tensor.matmul(yt_of[h][:], cur["v"][:, h, t, :],
                                         exp_sb[:, i, :],
                                         start=(t == 0), stop=(t == KT - 1))
                    pa = acc_p.tile([128, SC], bf16, tag="pa", name="pa")
                    nc.vector.tensor_tensor(out=pa[:], in0=exp_sb[:, 0, :],
                                            in1=exp_sb[:, 1, :], op=OP.add)
                    pacc_of[h].append(pa)
                    if g == NG - 1:
                        finish_head(h)

                def finish_head(h):
                    # bf16 pairwise tree over the 8 per-group partials;
                    # rounding is crushed by the exact f32 partition-sum
                    # matmul below. First level runs on the idle Pool.
                    pacc = pacc_of[h]
                    lvl = 0
                    while len(pacc) > 1:
                        nxt_l = []
                        for j in range(0, len(pacc), 2):
                            s = acc_p.tile([128, SC], bf16, tag="tr",
                                           name="tr")
                            eng = nc.gpsimd if lvl == 0 else nc.vector
                            eng.tensor_tensor(out=s[:], in0=pacc[j][:],
                                              in1=pacc[j + 1][:], op=OP.add)
                            nxt_l.append(s)
                        pacc = nxt_l
                        lvl += 1
                    den_ps = ps_den.tile([128, SC], f32, tag="den", name="den")
                    nc.tensor.matmul(den_ps[:], ones_b[:], pacc[0][:],
                                     start=True, stop=True)
                    # two DVE ops: yt and den are both PSUM and DVE can
                    # only read one PSUM operand per instruction
                    recd = rec_p.tile([128, SC], f32, tag="recd", name="recd")
                    nc.vector.reciprocal_approx_fast(out=recd[:],
                                                     in_=den_ps[:])
                    nc.vector.tensor_tensor(out=ynorm[:, h, :],
                                            in0=yt_of[h][:],
                                            in1=recd[:], op=OP.mult)

                def emit_att(kts_sb, h, g):
                    att_ps = ps_att.tile([128, 2, SC], f32, tag="att",
                                         name="att")
                    for i in range(2):
                        t = 2 * g + i
                        if t < KT - 1:
                            lhsT = kts_sb[:, h, t:t + 2, :]
                            rhs = qhat_sb[:, 2 * h + 1:2 * h + 3, :]
                        else:
                            lhsT = kts_sb[:, h, t - 1:t + 1, :]
                            rhs = qhat_sb[:, 2 * h:2 * h + 2, :]
                        nc.tensor.matmul(
                            att_ps[:, i, :], lhsT, rhs,
                            start=True, stop=True, perf_mode=DR)
                    return att_ps

                # attention matmuls are emitted one group AHEAD of their
                # exp so they sit in front of the y/chunk/proj bursts in
                # the PE's in-order queue -- the exp stream never starves.
                # The first two groups may have been prefetched by the
                # previous body (emitted right after its A01).
                att_tiles = dict(st.pop("att_pre", {}))
                ALLG = [(h, g) for h in range(H) for g in range(NG)]
                emit_ptr = [0]

                def ensure_att(upto):
                    while emit_ptr[0] <= min(upto, len(ALLG) - 1):
                        hh, gg = ALLG[emit_ptr[0]]
                        if (hh, gg) not in att_tiles:
                            att_tiles[(hh, gg)] = emit_att(cur["kts"], hh, gg)
                        emit_ptr[0] += 1

                for h in range(H):
                    yt_of[h] = ps_y.tile([128, SC], f32, tag="yt", name="yt")
                    for g in range(NG):
                        idx = h * NG + g
                        ensure_att(idx + 1)
                        att_ps = att_tiles.pop((h, g))
                        exp_sb = exp_p.tile([128, 2, SC], bf16, tag="exp",
                                            name="exp")
                        nc.scalar.activation(out=exp_sb[:], in_=att_ps[:],
                                             func=AF.Exp, bias=0.0, scale=1.0)
                        pend.append((h, g, exp_sb))
                        if len(pend) > 2:
                            consume(*pend.pop(0))
                        # emitted at g==2 (not g==0) so the Ln/Exp pair
                        # lands late enough in ACT's in-order stream that
                        # the proj/sq/ssq chain is ready when ACT gets there
                        if g == 2 and h + 2 < H:
                            emit_a(h + 2, cur["xt"], cur)
                        # previous rep's output projection, 2 accumulation
                        # steps per group in the PSUM idle window: groups
                        # g4..g7 per head, except chunk 7 runs at g0..g3
                        # (h7 has no A-phase holding the banks) to thin
                        # out the body-tail PE queue.
                        cg0 = 0 if h == H - 1 else NG - 4
                        if (not first) and cg0 <= g < cg0 + 4:
                            for k2 in range(2):
                                c_chunk_mm(h, (g - cg0) * 2 + k2,
                                           yprev, woprev)
                if not last:
                    # next rep's first two heads: emitted BEFORE the pend
                    # drain so their DVE ops (sq, qhat-mult) aren't queued
                    # behind the h7 pa/tree/div tail -- the first exp of
                    # the next body waits on qhat(h0). The drained h7 work
                    # has slack (its ynorm is only read mid-next-body).
                    emit_a(0, nxt["xt"], nxt, qt_pool=ps_qt, ssq_pool=ps_den)
                    emit_a(1, nxt["xt"], nxt, qt_pool=ps_sq, ssq_pool=ps_den)
                    do_load_wq27(nxt["wq27"])
                    # prefetch the next body's first two attention groups:
                    # the PE computes them during this body's drain so the
                    # next body's exp stream starts without a gap
                    st["att_pre"] = {
                        (0, g): emit_att(nxt["kts"], 0, g) for g in (0, 1)}
                while pend:
                    consume(*pend.pop(0))
                if last:
                    # iteration flush: this rep's output projection runs
                    # serially (the For_i back edge is an all-engine
                    # barrier + semaphore reset, so no tile dataflow may
                    # cross it -- amortized over the bodies per iteration)
                    for j in range(H):
                        for hh in range(H):
                            c_chunk_mm(j, hh, ynorm, cur["wo"])

                st["loads"] = nxt
                st["ynorm_prev"] = ynorm

            def prologue():
                st["loads"] = do_loads(cold=True)   # includes wq27
                emit_a(0, st["loads"]["xt"], st["loads"])
                emit_a(1, st["loads"]["xt"], st["loads"])

            def iteration(nbody):
                prologue()
                for j in range(nbody):
                    body(first=(j == 0), last=(j == nbody - 1))

            UNROLL = 8
            if reps > 1:
                n_iter, rem = divmod(reps - 1, UNROLL)
                assert rem == 0, f"timing reps must be {UNROLL}*k+1"
                with tc.For_i(0, n_iter, 1):
                    iteration(UNROLL)
                iteration(1)
            elif reps < 0:
                iteration(-reps)   # python-unrolled, TimelineSim only
            else:
                iteration(1)

    nc.compile()
    return nc


_CACHE = {}


def _get_nc(neg_heads=(), reps=1):
    # neg_heads kept for test.py interface compat; the attn_scale sign is
    # folded into the key bank on host so the device kernel never needs it.
    key = reps
    if key not in _CACHE:
        _CACHE[key] = build_nc(reps)
    return _CACHE[key]


def _make_in_maps(x, Wq, keys, values, attn_scale, Wo):
    x = np.asarray(x, dtype=np.float32)
    Wq = np.asarray(Wq, dtype=np.float32)
    Wo = np.asarray(Wo, dtype=np.float32)
    keys = np.asarray(keys, dtype=np.float32)
    values = np.asarray(values, dtype=np.float32)
    attn_scale = np.asarray(attn_scale, dtype=np.float32)

    # xT8[p, m, s(global)] = x[s, m*128+p]
    xT_all = x.reshape(S, D).T.reshape(MT, 128, S).transpose(1, 0, 2)
    xT8 = np.ascontiguousarray(xT_all).astype(NP_F8)
    # wq8[p, m, n] = Wq[m*128+p, n]
    wq8 = np.ascontiguousarray(
        Wq.reshape(MT, 128, D).transpose(1, 0, 2)).astype(NP_F8)
    # normalized+scaled keys, transposed + fp8: kts8[p(d), h, t, i]
    k3 = keys.reshape(K, H, HD)
    k3 = k3 * (attn_scale.reshape(1, H, 1) /
               np.sqrt((k3 * k3).sum(axis=-1, keepdims=True) + EPS))
    kts8 = np.ascontiguousarray(
        k3.transpose(2, 1, 0).reshape(HD, H, KT, 128)).astype(NP_F8)
    # v16[p(k%128), h, t, d] = values[t*128 + p, h, d]
    v3 = values.reshape(KT, 128, H, HD).transpose(1, 2, 0, 3)
    v16 = np.ascontiguousarray(v3).astype(NP_BF16)
    # wo16[p, h, o] = Wo[h*128+p, o]
    wo16 = np.ascontiguousarray(
        Wo.reshape(H, 128, D).transpose(1, 0, 2)).astype(NP_BF16)

    in_maps = []
    for c in range(N_CORES):
        in_maps.append({
            "xT": np.ascontiguousarray(xT8[:, :, c * SC:(c + 1) * SC]),
            "Wq": wq8, "kts": kts8, "v": v16, "Wo": wo16,
        })
    return in_maps


def kernel(x, Wq, keys, values, attn_scale, Wo):
    nc = _get_nc((), reps=1)
    in_maps = _make_in_maps(x, Wq, keys, values, attn_scale, Wo)
    res = run_bass_kernel_spmd(nc, in_maps, list(range(N_CORES)))
    out = np.concatenate([r["out"] for r in res.results], axis=0)
    return out.reshape(B, S, D).astype(np.float32)


# revision 39
# speedup vs baseline: 1.5296x; 1.5296x over previous
"""Trainium2 Bass kernel for nn_MultiHeadMLP (multi-head attention over a fixed
memory bank of 2048 slots/head, with L2-normalized queries/keys).

Sharding: data-parallel over the 4096-token sequence across 8 NeuronCores
(512 rows each); keys/values/projections replicated. No collectives.

Weight preprocessing on host (normalize+scale keys incl. attn_scale sign,
transposes, dtype casts); the device does all x-dependent compute.

v2: two structural changes over the 124us baseline.

1. fp8e4 DoubleRow attention-score matmuls. The attention contraction is
   only d=128, so DoubleRow (which wants 2x128 paired rows) is fed a
   second, zeroed pair-plane: qhat lives as [128, H, 3, SC] f8 with
   planes 0/2 zero and plane 1 the data; matmul t pairs kts tiles
   (t, t+1) with qhat planes (1, 2) = (data, zero), and the last tile
   t=15 pairs kts (14, 15) with planes (0, 1) = (zero, data) so no kts
   padding is needed. Halves the PE cost of the biggest matmul; a numpy
   error study shows fp8 q/k moves rel-err only 0.0067 -> 0.0084
   (softmax renormalization crushes pre-softmax quantization noise;
   fp8 on E/v/Wo would be 2.6-3.9e-2 and those stay bf16).

2. Software-pipelined rep loop. The trace showed a 27us ACT idle gap per
   rep: the output projection C ran as a serial PE block at rep end, then
   the next rep's query pipeline had to restart before the first exp.
   Now each body() iteration runs: B(i) (attention, ACT-paced by exp)
   with C(i-1)'s matmuls injected 2-per-group into B's PE slack (using
   the qt/ssq PSUM banks in their idle g4..g7 windows), loads for rep
   i+1 issued up front, and A(h0,h1) of rep i+1 emitted at body end.
   Prologue runs loads(0)+A01(0); epilogue runs the final C. For the
   first body, C(-1) reads a zeroed ynorm buffer and its stores are
   overwritten by the epilogue on the same DMA queue.

Engine budget per rep per core (cost-model):
  ACT ~77.5us busy = exp(att) 64x[128,2,512] + rstd Ln/Exp  <- pacer
  PE  ~66us busy = proj-DR + attn-DR + y-bf16 + ssq + den + Wo-bf16
  DVE ~50us: sq, qhat-f8 cast-mult, pairwise E adds, ynorm divide, out
  staging; Pool: lvl-0 E adds + store DMA issue.
DMA queues: x/Wq/Wo on sync, kts/v on the scalar queue, stores on gpsimd.

Single-activation-table design: ACT uses only Ln / Exp (one shared table;
rstd = exp(-0.5*ln(ssq+eps))) -> zero table reloads.
"""
import numpy as np
import ml_dtypes

import concourse.bacc as bacc
import concourse.mybir as mybir
import concourse.tile as tile
from concourse.bass_utils import run_bass_kernel_spmd

# The stock activation-table assigner first-fits Exp into 'exp_and_others',
# which lacks Ln, so a kernel using Ln+Exp ping-pongs tables (1.3us per
# reload). Hide those functions from every other table (keeping table
# order/IDs intact -- act_func_set_id is positional) so all of this
# kernel's activations land on 'natural_log_exp_and_others': zero reloads.
import concourse.hw_specs as _hw_specs
_ORIG_GAT = _hw_specs.get_activation_tables
_OUR_FNS = None  # set below after AF is defined


def _patched_gat(arch):
    tabs = _ORIG_GAT(arch)
    keep = "natural_log_exp_and_others"
    if keep not in tabs or not _OUR_FNS <= tabs[keep]:
        return tabs
    return {name: (fns if name == keep else fns - _OUR_FNS)
            for name, fns in tabs.items()}


_hw_specs.get_activation_tables = _patched_gat
bacc.get_activation_tables = _patched_gat

B, S, D = 1, 4096, 1024
H, HD, K = 8, 128, 2048
EPS = 1e-6
N_CORES = 8
SC = S // N_CORES      # 512 sequence rows per core
KT = K // 128          # 16 key tiles per head
MT = D // 128          # 8 contraction tiles for D
NG = KT // 2           # 8 attention groups per head
f32 = mybir.dt.float32
bf16 = mybir.dt.bfloat16
f8 = mybir.dt.float8e4
AF = mybir.ActivationFunctionType
OP = mybir.AluOpType
DR = mybir.MatmulPerfMode.DoubleRow
NP_F8 = ml_dtypes.float8_e4m3
NP_BF16 = ml_dtypes.bfloat16
_OUR_FNS = {AF.Exp, AF.Square, AF.Ln}


def build_nc(reps=1):
    nc = bacc.Bacc("TRN2", target_bir_lowering=False, debug=False, num_devices=N_CORES)
    xT = nc.dram_tensor("xT", [128, MT, SC], f8, kind="ExternalInput").ap()
    Wq = nc.dram_tensor("Wq", [128, MT, D], f8, kind="ExternalInput").ap()
    kts = nc.dram_tensor("kts", [128, H, KT, 128], f8, kind="ExternalInput").ap()
    v = nc.dram_tensor("v", [128, H, KT, HD], bf16, kind="ExternalInput").ap()
    Wo = nc.dram_tensor("Wo", [128, H, D], bf16, kind="ExternalInput").ap()
    out = nc.dram_tensor("out", [SC, D], f32, kind="ExternalOutput").ap()

    with tile.TileContext(nc) as tc:
        with tc.tile_pool(name="consts", bufs=1) as consts, \
             tc.tile_pool(name="wq01_p", bufs=2) as wq01_p, \
             tc.tile_pool(name="wq27_p", bufs=1) as wq27_p, \
             tc.tile_pool(name="xt_p", bufs=2) as xt_p, \
             tc.tile_pool(name="kts_p", bufs=2) as kts_p, \
             tc.tile_pool(name="v_p", bufs=2) as v_p, \
             tc.tile_pool(name="wo_p", bufs=2) as wo_p, \
             tc.tile_pool(name="ynorm_p", bufs=2) as ynorm_p, \
             tc.tile_pool(name="ps_att", bufs=2, space="PSUM") as ps_att, \
             tc.tile_pool(name="ps_y", bufs=1, space="PSUM") as ps_y, \
             tc.tile_pool(name="ps_qt", bufs=1, space="PSUM") as ps_qt, \
             tc.tile_pool(name="ps_sq", bufs=1, space="PSUM") as ps_sq, \
             tc.tile_pool(name="ps_den", bufs=1, space="PSUM") as ps_den, \
             tc.tile_pool(name="sqtmp", bufs=2) as sqtmp, \
             tc.tile_pool(name="qtb_p", bufs=2) as qtb_p, \
             tc.tile_pool(name="rec_p", bufs=2) as rec_p, \
             tc.tile_pool(name="exp_p", bufs=4) as exp_p, \
             tc.tile_pool(name="acc_p", bufs=5) as acc_p, \
             tc.tile_pool(name="outsb", bufs=1) as outsb:

            # constants: initialized once, shared by every rep
            eps_t = consts.tile([128, 1], f32)
            nc.vector.memset(eps_t[:], EPS)
            ones_f = consts.tile([128, 128], f32)
            nc.vector.memset(ones_f[:], 1.0)
            ones_b = consts.tile([128, 128], bf16)
            nc.vector.tensor_copy(out=ones_b[:], in_=ones_f[:])
            # qhat with interleaved zero planes for the DoubleRow attention
            # trick: plane 2h+1 holds head h's data, even planes stay zero
            # forever (plane 16 is the trailing zero for h=7).
            qhat_sb = consts.tile([128, 2 * H + 1, SC], f8)
            for zp in range(0, 2 * H + 1, 2):
                nc.vector.memset(qhat_sb[:, zp, :], 0.0)

            st = {}   # cross-emission ring state

            def do_loads(cold=False):
                """Issue the full set of input loads; returns tile handles.

                wq is split: heads 0/1 (used by the next-rep A01 at body
                end) are double-buffered; heads 2..7 single-buffered and
                loaded last on the sync queue so their buffer-free wait
                (last A7 read, ~60% into the previous body) blocks nothing
                else."""
                xt_sb = xt_p.tile([128, MT, SC], f8, name="xt_sb")
                wq01_sb = wq01_p.tile([128, MT, 256], f8, name="wq01_sb")
                wq27_sb = wq27_p.tile([128, MT, 768], f8, name="wq27_sb")
                kts_sb = kts_p.tile([128, H, KT, 128], f8, name="kts_sb")
                v_sb = v_p.tile([128, H, KT, HD], bf16, name="v_sb")
                wo_sb = wo_p.tile([128, H, D], bf16, name="wo_sb")
                # All loads on the SP queue (it has no compute to stall;
                # a trigger waiting on buffer-free blocks nothing).
                # cold=True (iteration prologue, after the For_i barrier):
                # strictly earliest-needed-first so the A01 chain and the
                # first attention groups start ASAP. Steady bodies load a
                # full rep ahead, so order barely matters there.
                if cold:
                    nc.sync.dma_start(out=wq01_sb[:], in_=Wq[:, :, 0:256])
                    for t in range(MT // 2):
                        nc.sync.dma_start(out=xt_sb[:, 2 * t:2 * t + 2, :],
                                          in_=xT[:, 2 * t:2 * t + 2, :])
                    nc.sync.dma_start(out=kts_sb[:, 0, :, :],
                                      in_=kts[:, 0, :, :])
                    nc.sync.dma_start(out=v_sb[:, 0, :, :], in_=v[:, 0, :, :])
                    for hp in range(3):
                        nc.sync.dma_start(
                            out=wq27_sb[:, :, hp * 256:(hp + 1) * 256],
                            in_=Wq[:, :, 256 + hp * 256:256 + (hp + 1) * 256])
                    for h in range(1, H):
                        nc.sync.dma_start(out=kts_sb[:, h, :, :],
                                          in_=kts[:, h, :, :])
                        nc.sync.dma_start(out=v_sb[:, h, :, :],
                                          in_=v[:, h, :, :])
                    nc.sync.dma_start(out=wo_sb[:], in_=Wo)
                else:
                    for h in range(H):
                        nc.sync.dma_start(out=kts_sb[:, h, :, :],
                                          in_=kts[:, h, :, :])
                        nc.sync.dma_start(out=v_sb[:, h, :, :],
                                          in_=v[:, h, :, :])
                    nc.sync.dma_start(out=wq01_sb[:], in_=Wq[:, :, 0:256])
                    for t in range(MT // 2):
                        nc.sync.dma_start(out=xt_sb[:, 2 * t:2 * t + 2, :],
                                          in_=xT[:, 2 * t:2 * t + 2, :])
                    nc.sync.dma_start(out=wo_sb[:], in_=Wo)
                return {"xt": xt_sb, "wq01": wq01_sb, "wq27": wq27_sb,
                        "kts": kts_sb, "v": v_sb, "wo": wo_sb}

            def do_load_wq27(wq27_sb):
                # single-buffered: must be EMITTED after this body's last
                # A-rest read of the previous epoch, else the tile
                # framework's same-buffer epoch ordering deadlocks the
                # hardware loop (load waits readers, readers wait load).
                for hp in range(3):
                    nc.sync.dma_start(
                        out=wq27_sb[:, :, hp * 256:(hp + 1) * 256],
                        in_=Wq[:, :, 256 + hp * 256:256 + (hp + 1) * 256])

            def emit_a(h, xt_sb, loads, qt_pool=None, ssq_pool=None):
                # query projection + normalization for head h -> qhat
                # data plane 2h+1. The A01 pair at body end passes split
                # pools so the two heads' chains overlap instead of
                # serializing through one PSUM bank.
                if h < 2:
                    wq_sb, col = loads["wq01"], h * 128
                else:
                    wq_sb, col = loads["wq27"], (h - 2) * 128
                qp, qtag = (qt_pool, "qt") if qt_pool is not None else (ps_qt, "qt")
                if qt_pool is ps_sq:
                    qtag = "ssq"
                sp, stag = (ssq_pool, "ssq") if ssq_pool is not None else (ps_sq, "ssq")
                if ssq_pool is ps_den:
                    stag = "den"
                qt_ps = qp.tile([128, SC], f32, tag=qtag, name="qt")
                for t in range(MT // 2):
                    nc.tensor.matmul(
                        qt_ps[:],
                        wq_sb[:, 2 * t:2 * t + 2, col:col + 128],
                        xt_sb[:, 2 * t:2 * t + 2, :],
                        start=(t == 0), stop=(t == MT // 2 - 1),
                        perf_mode=DR)
                # DVE cannot read two PSUM operands in one instruction, so
                # stage qt into SBUF bf16 first (also frees the qt PSUM
                # bank early and lets the square run in the fast 2x mode).
                qtb = qtb_p.tile([128, SC], bf16, tag="qtb", name="qtb")
                nc.vector.tensor_copy(out=qtb[:], in_=qt_ps[:])
                sq = sqtmp.tile([128, SC], bf16, tag="sq", name="sq")
                nc.vector.tensor_tensor(out=sq[:], in0=qtb[:],
                                        in1=qtb[:], op=OP.mult)
                ssq_ps = sp.tile([128, SC], f32, tag=stag, name="ssq")
                nc.tensor.matmul(ssq_ps[:], ones_b[:], sq[:],
                                 start=True, stop=True)
                lns = sqtmp.tile([128, SC], f32, tag="lns", name="lns")
                nc.scalar.activation(out=lns[:], in_=ssq_ps[:],
                                     func=AF.Ln, bias=eps_t[:], scale=1.0)
                rstd = sqtmp.tile([128, SC], f32, tag="rstd", name="rstd")
                nc.scalar.activation(out=rstd[:], in_=lns[:],
                                     func=AF.Exp, bias=0.0, scale=-0.5)
                nc.vector.tensor_tensor(out=qhat_sb[:, 2 * h + 1, :],
                                        in0=qtb[:], in1=rstd[:], op=OP.mult)

            def c_chunk_mm(j, hh, yn, wo_sb):
                # output-projection accumulation step hh of chunk j
                si, oc = j // 2, j % 2
                pool, tg = (ps_qt, "qt") if j % 2 == 0 else (ps_sq, "ssq")
                if hh == 0:
                    st[("ops", j)] = pool.tile([128, SC], f32, tag=tg,
                                               name="ops")
                o_ps = st[("ops", j)]
                nc.tensor.matmul(
                    o_ps[:],
                    yn[:, hh, si * 128:(si + 1) * 128],
                    wo_sb[:, hh, oc * 512:(oc + 1) * 512],
                    start=(hh == 0), stop=(hh == H - 1))
                if hh == H - 1:
                    o_sb = outsb.tile([128, 512], f32, tag="osb", name="osb")
                    nc.vector.tensor_copy(out=o_sb[:], in_=o_ps[:])
                    nc.gpsimd.dma_start(
                        out=out[si * 128:(si + 1) * 128,
                                oc * 512:(oc + 1) * 512],
                        in_=o_sb[:])

            def body(first=False, last=False):
                cur = st["loads"]
                nxt = None if last else do_loads()
                ynorm = ynorm_p.tile([128, H, SC], bf16, name="ynorm")
                # C chunks read the CURRENT wo buffer (same values as the
                # previous rep's -- Wo reloads identically every rep) so
                # the wo load pipeline never waits on late chunk readers.
                yprev, woprev = st.get("ynorm_prev"), cur["wo"]

                yt_of = {}
                pacc_of = {h: [] for h in range(H)}
                pend = []   # two-step lag queue of (h, g, exp_sb)

                def consume(h, g, exp_sb):
                    for i in range(2):
                        t = 2 * g + i
                        nc.tensor.matmul(yt_of[h][:], cur["v"][:, h, t, :],
                                         exp_sb[:, i, :],
                                         start=(t == 0), stop=(t == KT - 1))
                    pa = acc_p.tile([128, SC], bf16, tag="pa", name="pa")
                    nc.vector.tensor_tensor(out=pa[:], in0=exp_sb[:, 0, :],
                                            in1=exp_sb[:, 1, :], op=OP.add)
                    pacc_of[h].append(pa)
                    if g == NG - 1:
                        finish_head(h)

                def finish_head(h):
                    # bf16 pairwise tree over the 8 per-group partials;
                    # rounding is crushed by the exact f32 partition-sum
                    # matmul below. First level runs on the idle Pool.
                    pacc = pacc_of[h]
                    lvl = 0
                    while len(pacc) > 1:
                        nxt_l = []
                        for j in range(0, len(pacc), 2):
                            s = acc_p.tile([128, SC], bf16, tag="tr",
                                           name="tr")
                            eng = nc.gpsimd if lvl == 0 else nc.vector
                            eng.tensor_tensor(out=s[:], in0=pacc[j][:],
                                              in1=pacc[j + 1][:], op=OP.add)
                            nxt_l.append(s)
                        pacc = nxt_l
                        lvl += 1
                    den_ps = ps_den.tile([128, SC], f32, tag="den", name="den")
                    nc.tensor.matmul(den_ps[:], ones_b[:], pacc[0][:],
                                     start=True, stop=True)
                    # two DVE ops: yt and den are both PSUM and DVE can
                    # only read one PSUM operand per instruction
                    recd = rec_p.tile([128, SC], f32, tag="recd", name="recd")
                    nc.vector.reciprocal_approx_fast(out=recd[:],
                                                     in_=den_ps[:])
                    nc.vector.tensor_tensor(out=ynorm[:, h, :],
                                            in0=yt_of[h][:],
                                            in1=recd[:], op=OP.mult)

                def emit_att(kts_sb, h, g):
                    att_ps = ps_att.tile([128, 2, SC], f32, tag="att",
                                         name="att")
                    for i in range(2):
                        t = 2 * g + i
                        if t < KT - 1:
                            lhsT = kts_sb[:, h, t:t + 2, :]
                            rhs = qhat_sb[:, 2 * h + 1:2 * h + 3, :]
                        else:
                            lhsT = kts_sb[:, h, t - 1:t + 1, :]
                            rhs = qhat_sb[:, 2 * h:2 * h + 2, :]
                        nc.tensor.matmul(
                            att_ps[:, i, :], lhsT, rhs,
                            start=True, stop=True, perf_mode=DR)
                    return att_ps

                for h in range(H):
                    yt_of[h] = ps_y.tile([128, SC], f32, tag="yt", name="yt")
                    for g in range(NG):
                        att_ps = emit_att(cur["kts"], h, g)
                        exp_sb = exp_p.tile([128, 2, SC], bf16, tag="exp",
                                            name="exp")
                        nc.scalar.activation(out=exp_sb[:], in_=att_ps[:],
                                             func=AF.Exp, bias=0.0, scale=1.0)
                        pend.append((h, g, exp_sb))
                        if len(pend) > 2:
                            consume(*pend.pop(0))
                        # emitted at g==2 (not g==0) so the Ln/Exp pair
                        # lands late enough in ACT's in-order stream that
                        # the proj/sq/ssq chain is ready when ACT gets there
                        if g == 2 and h + 2 < H:
                            emit_a(h + 2, cur["xt"], cur)
                        # previous rep's output projection, 2 accumulation
                        # steps per group in the PSUM idle window: groups
                        # g4..g7 per head, except chunk 7 runs at g0..g3
                        # (h7 has no A-phase holding the banks) to thin
                        # out the body-tail PE queue.
                        cg0 = 0 if h == H - 1 else NG - 4
                        if (not first) and cg0 <= g < cg0 + 4:
                            for k2 in range(2):
                                c_chunk_mm(h, (g - cg0) * 2 + k2,
                                           yprev, woprev)
                if not last:
                    # next rep's first two heads: emitted BEFORE the pend
                    # drain so their DVE ops (sq, qhat-mult) aren't queued
                    # behind the h7 pa/tree/div tail -- the first exp of
                    # the next body waits on qhat(h0). The drained h7 work
                    # has slack (its ynorm is only read mid-next-body).
                    emit_a(0, nxt["xt"], nxt, qt_pool=ps_qt, ssq_pool=ps_den)
                    emit_a(1, nxt["xt"], nxt, qt_pool=ps_sq, ssq_pool=ps_den)
                    do_load_wq27(nxt["wq27"])
                while pend:
                    consume(*pend.pop(0))
                if last:
                    # iteration flush: this rep's output projection runs
                    # serially (the For_i back edge is an all-engine
                    # barrier + semaphore reset, so no tile dataflow may
                    # cross it -- amortized over the bodies per iteration)
                    for j in range(H):
                        for hh in range(H):
                            c_chunk_mm(j, hh, ynorm, cur["wo"])

                st["loads"] = nxt
                st["ynorm_prev"] = ynorm

            def prologue():
                st["loads"] = do_loads(cold=True)   # includes wq27
                emit_a(0, st["loads"]["xt"], st["loads"])
                emit_a(1, st["loads"]["xt"], st["loads"])

            def iteration(nbody):
                prologue()
                for j in range(nbody):
                    body(first=(j == 0), last=(j == nbody - 1))

            UNROLL = 8
            if reps > 1:
                n_iter, rem = divmod(reps - 1, UNROLL)
                assert rem == 0, f"timing reps must be {UNROLL}*k+1"
                with tc.For_i(0, n_iter, 1):
                    iteration(UNROLL)
                iteration(1)
            elif reps < 0:
                iteration(-reps)   # python-unrolled, TimelineSim only
            else:
                iteration(1)

    nc.compile()
    return nc


_CACHE = {}


def _get_nc(neg_heads=(), reps=1):
    # neg_heads kept for test.py interface compat; the attn_scale sign is
    # folded into the key bank on host so the device kernel never needs it.
    key = reps
    if key not in _CACHE:
        _CACHE[key] = build_nc(reps)
    return _CACHE[key]


def _make_in_maps(x, Wq, keys, values, attn_scale, Wo):
    x = np.asarray(x, dtype=np.float32)
    Wq = np.asarray(Wq, dtype=np.float32)
    Wo = np.asarray(Wo, dtype=np.float32)
    keys = np.asarray(keys, dtype=np.float32)
    values = np.asarray(values, dtype=np.float32)
    attn_scale = np.asarray(attn_scale, dtype=np.float32)

    # xT8[p, m, s(global)] = x[s, m*128+p]
    xT_all = x.reshape(S, D).T.reshape(MT, 128, S).transpose(1, 0, 2)
    xT8 = np.ascontiguousarray(xT_all).astype(NP_F8)
    # wq8[p, m, n] = Wq[m*128+p, n]
    wq8 = np.ascontiguousarray(
        Wq.reshape(MT, 128, D).transpose(1, 0, 2)).astype(NP_F8)
    # normalized+scaled keys, transposed + fp8: kts8[p(d), h, t, i]
    k3 = keys.reshape(K, H, HD)
    k3 = k3 * (attn_scale.reshape(1, H, 1) /
               np.sqrt((k3 * k3).sum(axis=-1, keepdims=True) + EPS))
    kts8 = np.ascontiguousarray(
        k3.transpose(2, 1, 0).reshape(HD, H, KT, 128)).astype(NP_F8)
    # v16[p(k%128), h, t, d] = values[t*128 + p, h, d]
    v3 = values.reshape(KT, 128, H, HD).transpose(1, 2, 0, 3)
    v16 = np.ascontiguousarray(v3).astype(NP_BF16)
    # wo16[p, h, o] = Wo[h*128+p, o]
    wo16 = np.ascontiguousarray(
        Wo.reshape(H, 128, D).transpose(1, 0, 2)).astype(NP_BF16)

    in_maps = []
    for c in range(N_CORES):
        in_maps.append({
            "xT": np.ascontiguousarray(xT8[:, :, c * SC:(c + 1) * SC]),
            "Wq": wq8, "kts": kts8, "v": v16, "Wo": wo16,
        })
    return in_maps


def kernel(x, Wq, keys, values, attn_scale, Wo):
    nc = _get_nc((), reps=1)
    in_maps = _make_in_maps(x, Wq, keys, values, attn_scale, Wo)
    res = run_bass_kernel_spmd(nc, in_maps, list(range(N_CORES)))
    out = np.concatenate([r["out"] for r in res.results], axis=0)
    return out.reshape(B, S, D).astype(np.float32)


# revision 40
# speedup vs baseline: 1.5780x; 1.0316x over previous
"""Trainium2 Bass kernel for nn_MultiHeadMLP (multi-head attention over a fixed
memory bank of 2048 slots/head, with L2-normalized queries/keys).

Sharding: data-parallel over the 4096-token sequence across 8 NeuronCores
(512 rows each); keys/values/projections replicated. No collectives.

Weight preprocessing on host (normalize+scale keys incl. attn_scale sign,
transposes, dtype casts); the device does all x-dependent compute.

v2: two structural changes over the 124us baseline.

1. fp8e4 DoubleRow attention-score matmuls. The attention contraction is
   only d=128, so DoubleRow (which wants 2x128 paired rows) is fed a
   second, zeroed pair-plane: qhat lives as [128, H, 3, SC] f8 with
   planes 0/2 zero and plane 1 the data; matmul t pairs kts tiles
   (t, t+1) with qhat planes (1, 2) = (data, zero), and the last tile
   t=15 pairs kts (14, 15) with planes (0, 1) = (zero, data) so no kts
   padding is needed. Halves the PE cost of the biggest matmul; a numpy
   error study shows fp8 q/k moves rel-err only 0.0067 -> 0.0084
   (softmax renormalization crushes pre-softmax quantization noise;
   fp8 on E/v/Wo would be 2.6-3.9e-2 and those stay bf16).

2. Software-pipelined rep loop. The trace showed a 27us ACT idle gap per
   rep: the output projection C ran as a serial PE block at rep end, then
   the next rep's query pipeline had to restart before the first exp.
   Now each body() iteration runs: B(i) (attention, ACT-paced by exp)
   with C(i-1)'s matmuls injected 2-per-group into B's PE slack (using
   the qt/ssq PSUM banks in their idle g4..g7 windows), loads for rep
   i+1 issued up front, and A(h0,h1) of rep i+1 emitted at body end.
   Prologue runs loads(0)+A01(0); epilogue runs the final C. For the
   first body, C(-1) reads a zeroed ynorm buffer and its stores are
   overwritten by the epilogue on the same DMA queue.

Engine budget per rep per core (cost-model):
  ACT ~77.5us busy = exp(att) 64x[128,2,512] + rstd Ln/Exp  <- pacer
  PE  ~66us busy = proj-DR + attn-DR + y-bf16 + ssq + den + Wo-bf16
  DVE ~50us: sq, qhat-f8 cast-mult, pairwise E adds, ynorm divide, out
  staging; Pool: lvl-0 E adds + store DMA issue.
DMA queues: x/Wq/Wo on sync, kts/v on the scalar queue, stores on gpsimd.

Single-activation-table design: ACT uses only Ln / Exp (one shared table;
rstd = exp(-0.5*ln(ssq+eps))) -> zero table reloads.
"""
import numpy as np
import ml_dtypes

import concourse.bacc as bacc
import concourse.mybir as mybir
import concourse.tile as tile
from concourse.bass_utils import run_bass_kernel_spmd

# The stock activation-table assigner first-fits Exp into 'exp_and_others',
# which lacks Ln, so a kernel using Ln+Exp ping-pongs tables (1.3us per
# reload). Hide those functions from every other table (keeping table
# order/IDs intact -- act_func_set_id is positional) so all of this
# kernel's activations land on 'natural_log_exp_and_others': zero reloads.
import concourse.hw_specs as _hw_specs
_ORIG_GAT = _hw_specs.get_activation_tables
_OUR_FNS = None  # set below after AF is defined


def _patched_gat(arch):
    tabs = _ORIG_GAT(arch)
    keep = "natural_log_exp_and_others"
    if keep not in tabs or not _OUR_FNS <= tabs[keep]:
        return tabs
    return {name: (fns if name == keep else fns - _OUR_FNS)
            for name, fns in tabs.items()}


_hw_specs.get_activation_tables = _patched_gat
bacc.get_activation_tables = _patched_gat

B, S, D = 1, 4096, 1024
H, HD, K = 8, 128, 2048
EPS = 1e-6
N_CORES = 8
SC = S // N_CORES      # 512 sequence rows per core
KT = K // 128          # 16 key tiles per head
MT = D // 128          # 8 contraction tiles for D
NG = KT // 2           # 8 attention groups per head
f32 = mybir.dt.float32
bf16 = mybir.dt.bfloat16
f8 = mybir.dt.float8e4
AF = mybir.ActivationFunctionType
OP = mybir.AluOpType
DR = mybir.MatmulPerfMode.DoubleRow
NP_F8 = ml_dtypes.float8_e4m3
NP_BF16 = ml_dtypes.bfloat16
_OUR_FNS = {AF.Exp, AF.Square, AF.Ln}


def build_nc(reps=1):
    nc = bacc.Bacc("TRN2", target_bir_lowering=False, debug=False, num_devices=N_CORES)
    xT = nc.dram_tensor("xT", [128, MT, SC], f8, kind="ExternalInput").ap()
    Wq = nc.dram_tensor("Wq", [128, MT, D], f8, kind="ExternalInput").ap()
    kts = nc.dram_tensor("kts", [128, H, KT, 128], f8, kind="ExternalInput").ap()
    v = nc.dram_tensor("v", [128, H, KT, HD], bf16, kind="ExternalInput").ap()
    Wo = nc.dram_tensor("Wo", [128, H, D], bf16, kind="ExternalInput").ap()
    out = nc.dram_tensor("out", [SC, D], f32, kind="ExternalOutput").ap()

    with tile.TileContext(nc) as tc:
        with tc.tile_pool(name="consts", bufs=1) as consts, \
             tc.tile_pool(name="wq01_p", bufs=2) as wq01_p, \
             tc.tile_pool(name="wq27_p", bufs=1) as wq27_p, \
             tc.tile_pool(name="xt_p", bufs=2) as xt_p, \
             tc.tile_pool(name="kts_p", bufs=2) as kts_p, \
             tc.tile_pool(name="v_p", bufs=2) as v_p, \
             tc.tile_pool(name="wo_p", bufs=2) as wo_p, \
             tc.tile_pool(name="ynorm_p", bufs=2) as ynorm_p, \
             tc.tile_pool(name="ps_att", bufs=2, space="PSUM") as ps_att, \
             tc.tile_pool(name="ps_y", bufs=1, space="PSUM") as ps_y, \
             tc.tile_pool(name="ps_qt", bufs=1, space="PSUM") as ps_qt, \
             tc.tile_pool(name="ps_sq", bufs=1, space="PSUM") as ps_sq, \
             tc.tile_pool(name="ps_den", bufs=1, space="PSUM") as ps_den, \
             tc.tile_pool(name="sqtmp", bufs=2) as sqtmp, \
             tc.tile_pool(name="qtb_p", bufs=2) as qtb_p, \
             tc.tile_pool(name="rec_p", bufs=2) as rec_p, \
             tc.tile_pool(name="exp_p", bufs=4) as exp_p, \
             tc.tile_pool(name="acc_p", bufs=5) as acc_p, \
             tc.tile_pool(name="outsb", bufs=1) as outsb:

            # constants: initialized once, shared by every rep
            eps_t = consts.tile([128, 1], f32)
            nc.vector.memset(eps_t[:], EPS)
            ones_f = consts.tile([128, 128], f32)
            nc.vector.memset(ones_f[:], 1.0)
            ones_b = consts.tile([128, 128], bf16)
            nc.vector.tensor_copy(out=ones_b[:], in_=ones_f[:])
            # qhat with interleaved zero planes for the DoubleRow attention
            # trick: plane 2h+1 holds head h's data, even planes stay zero
            # forever (plane 16 is the trailing zero for h=7).
            qhat_sb = consts.tile([128, 2 * H + 1, SC], f8)
            for zp in range(0, 2 * H + 1, 2):
                nc.vector.memset(qhat_sb[:, zp, :], 0.0)

            st = {}   # cross-emission ring state

            def do_loads(cold=False):
                """Issue the full set of input loads; returns tile handles.

                wq is split: heads 0/1 (used by the next-rep A01 at body
                end) are double-buffered; heads 2..7 single-buffered and
                loaded last on the sync queue so their buffer-free wait
                (last A7 read, ~60% into the previous body) blocks nothing
                else."""
                xt_sb = xt_p.tile([128, MT, SC], f8, name="xt_sb")
                wq01_sb = wq01_p.tile([128, MT, 256], f8, name="wq01_sb")
                wq27_sb = wq27_p.tile([128, MT, 768], f8, name="wq27_sb")
                kts_sb = kts_p.tile([128, H, KT, 128], f8, name="kts_sb")
                v_sb = v_p.tile([128, H, KT, HD], bf16, name="v_sb")
                wo_sb = wo_p.tile([128, H, D], bf16, name="wo_sb")
                # All loads on the SP queue (it has no compute to stall;
                # a trigger waiting on buffer-free blocks nothing).
                # cold=True (iteration prologue, after the For_i barrier):
                # strictly earliest-needed-first so the A01 chain and the
                # first attention groups start ASAP. Steady bodies load a
                # full rep ahead, so order barely matters there.
                if cold:
                    nc.sync.dma_start(out=wq01_sb[:], in_=Wq[:, :, 0:256])
                    for t in range(MT // 2):
                        nc.sync.dma_start(out=xt_sb[:, 2 * t:2 * t + 2, :],
                                          in_=xT[:, 2 * t:2 * t + 2, :])
                    nc.sync.dma_start(out=kts_sb[:, 0, :, :],
                                      in_=kts[:, 0, :, :])
                    nc.sync.dma_start(out=v_sb[:, 0, :, :], in_=v[:, 0, :, :])
                    for hp in range(3):
                        nc.sync.dma_start(
                            out=wq27_sb[:, :, hp * 256:(hp + 1) * 256],
                            in_=Wq[:, :, 256 + hp * 256:256 + (hp + 1) * 256])
                    for h in range(1, H):
                        nc.sync.dma_start(out=kts_sb[:, h, :, :],
                                          in_=kts[:, h, :, :])
                        nc.sync.dma_start(out=v_sb[:, h, :, :],
                                          in_=v[:, h, :, :])
                    nc.sync.dma_start(out=wo_sb[:], in_=Wo)
                else:
                    for h in range(H):
                        nc.sync.dma_start(out=kts_sb[:, h, :, :],
                                          in_=kts[:, h, :, :])
                        nc.sync.dma_start(out=v_sb[:, h, :, :],
                                          in_=v[:, h, :, :])
                    nc.sync.dma_start(out=wq01_sb[:], in_=Wq[:, :, 0:256])
                    for t in range(MT // 2):
                        nc.sync.dma_start(out=xt_sb[:, 2 * t:2 * t + 2, :],
                                          in_=xT[:, 2 * t:2 * t + 2, :])
                    nc.sync.dma_start(out=wo_sb[:], in_=Wo)
                return {"xt": xt_sb, "wq01": wq01_sb, "wq27": wq27_sb,
                        "kts": kts_sb, "v": v_sb, "wo": wo_sb}

            def do_load_wq27(wq27_sb):
                # single-buffered: must be EMITTED after this body's last
                # A-rest read of the previous epoch, else the tile
                # framework's same-buffer epoch ordering deadlocks the
                # hardware loop (load waits readers, readers wait load).
                for hp in range(3):
                    nc.sync.dma_start(
                        out=wq27_sb[:, :, hp * 256:(hp + 1) * 256],
                        in_=Wq[:, :, 256 + hp * 256:256 + (hp + 1) * 256])

            def emit_a(h, xt_sb, loads, qt_pool=None, ssq_pool=None):
                # query projection + normalization for head h -> qhat
                # data plane 2h+1. The A01 pair at body end passes split
                # pools so the two heads' chains overlap instead of
                # serializing through one PSUM bank.
                if h < 2:
                    wq_sb, col = loads["wq01"], h * 128
                else:
                    wq_sb, col = loads["wq27"], (h - 2) * 128
                qp, qtag = (qt_pool, "qt") if qt_pool is not None else (ps_qt, "qt")
                if qt_pool is ps_sq:
                    qtag = "ssq"
                sp, stag = (ssq_pool, "ssq") if ssq_pool is not None else (ps_sq, "ssq")
                if ssq_pool is ps_den:
                    stag = "den"
                qt_ps = qp.tile([128, SC], f32, tag=qtag, name="qt")
                for t in range(MT // 2):
                    nc.tensor.matmul(
                        qt_ps[:],
                        wq_sb[:, 2 * t:2 * t + 2, col:col + 128],
                        xt_sb[:, 2 * t:2 * t + 2, :],
                        start=(t == 0), stop=(t == MT // 2 - 1),
                        perf_mode=DR)
                # DVE cannot read two PSUM operands in one instruction, so
                # stage qt into SBUF bf16 first (also frees the qt PSUM
                # bank early and lets the square run in the fast 2x mode).
                qtb = qtb_p.tile([128, SC], bf16, tag="qtb", name="qtb")
                nc.vector.tensor_copy(out=qtb[:], in_=qt_ps[:])
                sq = sqtmp.tile([128, SC], bf16, tag="sq", name="sq")
                nc.vector.tensor_tensor(out=sq[:], in0=qtb[:],
                                        in1=qtb[:], op=OP.mult)
                ssq_ps = sp.tile([128, SC], f32, tag=stag, name="ssq")
                nc.tensor.matmul(ssq_ps[:], ones_b[:], sq[:],
                                 start=True, stop=True)
                lns = sqtmp.tile([128, SC], f32, tag="lns", name="lns")
                nc.scalar.activation(out=lns[:], in_=ssq_ps[:],
                                     func=AF.Ln, bias=eps_t[:], scale=1.0)
                rstd = sqtmp.tile([128, SC], f32, tag="rstd", name="rstd")
                nc.scalar.activation(out=rstd[:], in_=lns[:],
                                     func=AF.Exp, bias=0.0, scale=-0.5)
                nc.vector.tensor_tensor(out=qhat_sb[:, 2 * h + 1, :],
                                        in0=qtb[:], in1=rstd[:], op=OP.mult)

            def c_chunk_mm(j, hh, yn, wo_sb):
                # output-projection accumulation step hh of chunk j
                si, oc = j // 2, j % 2
                pool, tg = (ps_qt, "qt") if j % 2 == 0 else (ps_sq, "ssq")
                if hh == 0:
                    st[("ops", j)] = pool.tile([128, SC], f32, tag=tg,
                                               name="ops")
                o_ps = st[("ops", j)]
                nc.tensor.matmul(
                    o_ps[:],
                    yn[:, hh, si * 128:(si + 1) * 128],
                    wo_sb[:, hh, oc * 512:(oc + 1) * 512],
                    start=(hh == 0), stop=(hh == H - 1))
                if hh == H - 1:
                    o_sb = outsb.tile([128, 512], f32, tag="osb", name="osb")
                    nc.vector.tensor_copy(out=o_sb[:], in_=o_ps[:])
                    nc.gpsimd.dma_start(
                        out=out[si * 128:(si + 1) * 128,
                                oc * 512:(oc + 1) * 512],
                        in_=o_sb[:])

            def body(first=False, last=False):
                cur = st["loads"]
                nxt = None if last else do_loads()
                ynorm = ynorm_p.tile([128, H, SC], bf16, name="ynorm")
                # C chunks read the CURRENT wo buffer (same values as the
                # previous rep's -- Wo reloads identically every rep) so
                # the wo load pipeline never waits on late chunk readers.
                yprev, woprev = st.get("ynorm_prev"), cur["wo"]

                yt_of = {}
                pacc_of = {h: [] for h in range(H)}
                pend = []   # two-step lag queue of (h, g, exp_sb)

                def consume(h, g, exp_sb):
                    for i in range(2):
                        t = 2 * g + i
                        nc.tensor.matmul(yt_of[h][:], cur["v"][:, h, t, :],
                                         exp_sb[:, i, :],
                                         start=(t == 0), stop=(t == KT - 1))
                    pa = acc_p.tile([128, SC], bf16, tag="pa", name="pa")
                    nc.vector.tensor_tensor(out=pa[:], in0=exp_sb[:, 0, :],
                                            in1=exp_sb[:, 1, :], op=OP.add)
                    pacc_of[h].append(pa)
                    if g == NG - 1:
                        finish_head(h)

                def finish_head(h):
                    # bf16 pairwise tree over the 8 per-group partials;
                    # rounding is crushed by the exact f32 partition-sum
                    # matmul below. First level runs on the idle Pool.
                    pacc = pacc_of[h]
                    lvl = 0
                    while len(pacc) > 1:
                        nxt_l = []
                        for j in range(0, len(pacc), 2):
                            s = acc_p.tile([128, SC], bf16, tag="tr",
                                           name="tr")
                            eng = nc.gpsimd if lvl == 0 else nc.vector
                            eng.tensor_tensor(out=s[:], in0=pacc[j][:],
                                              in1=pacc[j + 1][:], op=OP.add)
                            nxt_l.append(s)
                        pacc = nxt_l
                        lvl += 1
                    den_ps = ps_den.tile([128, SC], f32, tag="den", name="den")
                    nc.tensor.matmul(den_ps[:], ones_b[:], pacc[0][:],
                                     start=True, stop=True)
                    # two DVE ops: yt and den are both PSUM and DVE can
                    # only read one PSUM operand per instruction
                    recd = rec_p.tile([128, SC], f32, tag="recd", name="recd")
                    nc.vector.reciprocal_approx_fast(out=recd[:],
                                                     in_=den_ps[:])
                    nc.vector.tensor_tensor(out=ynorm[:, h, :],
                                            in0=yt_of[h][:],
                                            in1=recd[:], op=OP.mult)

                def emit_att(kts_sb, h, g):
                    att_ps = ps_att.tile([128, 2, SC], f32, tag="att",
                                         name="att")
                    for i in range(2):
                        t = 2 * g + i
                        if t < KT - 1:
                            lhsT = kts_sb[:, h, t:t + 2, :]
                            rhs = qhat_sb[:, 2 * h + 1:2 * h + 3, :]
                        else:
                            lhsT = kts_sb[:, h, t - 1:t + 1, :]
                            rhs = qhat_sb[:, 2 * h:2 * h + 2, :]
                        nc.tensor.matmul(
                            att_ps[:, i, :], lhsT, rhs,
                            start=True, stop=True, perf_mode=DR)
                    return att_ps

                for h in range(H):
                    yt_of[h] = ps_y.tile([128, SC], f32, tag="yt", name="yt")
                    for g in range(NG):
                        att_ps = emit_att(cur["kts"], h, g)
                        exp_sb = exp_p.tile([128, 2, SC], bf16, tag="exp",
                                            name="exp")
                        nc.scalar.activation(out=exp_sb[:], in_=att_ps[:],
                                             func=AF.Exp, bias=0.0, scale=1.0)
                        pend.append((h, g, exp_sb))
                        if len(pend) > 2:
                            consume(*pend.pop(0))
                        # emitted at g==2 (not g==0) so the Ln/Exp pair
                        # lands late enough in ACT's in-order stream that
                        # the proj/sq/ssq chain is ready when ACT gets there
                        if g == 2 and h + 2 < H:
                            emit_a(h + 2, cur["xt"], cur)
                        # previous rep's output projection, 2 accumulation
                        # steps per group in the PSUM idle window: groups
                        # g4..g7 per head, except chunk 7 runs at g0..g3
                        # (h7 has no A-phase holding the banks) to thin
                        # out the body-tail PE queue.
                        cg0 = 0 if h == H - 1 else NG - 4
                        if (not first) and cg0 <= g < cg0 + 4:
                            for k2 in range(2):
                                c_chunk_mm(h, (g - cg0) * 2 + k2,
                                           yprev, woprev)
                if not last:
                    # next rep's first two heads: emitted BEFORE the pend
                    # drain so their DVE ops (sq, qhat-mult) aren't queued
                    # behind the h7 pa/tree/div tail -- the first exp of
                    # the next body waits on qhat(h0). The drained h7 work
                    # has slack (its ynorm is only read mid-next-body).
                    emit_a(0, nxt["xt"], nxt, qt_pool=ps_qt, ssq_pool=ps_den)
                    emit_a(1, nxt["xt"], nxt, qt_pool=ps_sq, ssq_pool=ps_den)
                    do_load_wq27(nxt["wq27"])
                while pend:
                    consume(*pend.pop(0))
                if last:
                    # iteration flush: this rep's output projection runs
                    # serially (the For_i back edge is an all-engine
                    # barrier + semaphore reset, so no tile dataflow may
                    # cross it -- amortized over the bodies per iteration)
                    for j in range(H):
                        for hh in range(H):
                            c_chunk_mm(j, hh, ynorm, cur["wo"])

                st["loads"] = nxt
                st["ynorm_prev"] = ynorm

            def prologue():
                st["loads"] = do_loads(cold=True)   # includes wq27
                emit_a(0, st["loads"]["xt"], st["loads"])
                emit_a(1, st["loads"]["xt"], st["loads"])

            def iteration(nbody):
                prologue()
                for j in range(nbody):
                    body(first=(j == 0), last=(j == nbody - 1))

            UNROLL = 16
            if reps > 1:
                n_iter, rem = divmod(reps - 1, UNROLL)
                assert rem == 0, f"timing reps must be {UNROLL}*k+1"
                with tc.For_i(0, n_iter, 1):
                    iteration(UNROLL)
                iteration(1)
            elif reps < 0:
                iteration(-reps)   # python-unrolled, TimelineSim only
            else:
                iteration(1)

    nc.compile()
    return nc


_CACHE = {}


def _get_nc(neg_heads=(), reps=1):
    # neg_heads kept for test.py interface compat; the attn_scale sign is
    # folded into the key bank on host so the device kernel never needs it.
    key = reps
    if key not in _CACHE:
        _CACHE[key] = build_nc(reps)
    return _CACHE[key]


def _make_in_maps(x, Wq, keys, values, attn_scale, Wo):
    x = np.asarray(x, dtype=np.float32)
    Wq = np.asarray(Wq, dtype=np.float32)
    Wo = np.asarray(Wo, dtype=np.float32)
    keys = np.asarray(keys, dtype=np.float32)
    values = np.asarray(values, dtype=np.float32)
    attn_scale = np.asarray(attn_scale, dtype=np.float32)

    # xT8[p, m, s(global)] = x[s, m*128+p]
    xT_all = x.reshape(S, D).T.reshape(MT, 128, S).transpose(1, 0, 2)
    xT8 = np.ascontiguousarray(xT_all).astype(NP_F8)
    # wq8[p, m, n] = Wq[m*128+p, n]
    wq8 = np.ascontiguousarray(
        Wq.reshape(MT, 128, D).transpose(1, 0, 2)).astype(NP_F8)
    # normalized+scaled keys, transposed + fp8: kts8[p(d), h, t, i]
    k3 = keys.reshape(K, H, HD)
    k3 = k3 * (attn_scale.reshape(1, H, 1) /
               np.sqrt((k3 * k3).sum(axis=-1, keepdims=True) + EPS))
    kts8 = np.ascontiguousarray(
        k3.transpose(2, 1, 0).reshape(HD, H, KT, 128)).astype(NP_F8)
    # v16[p(k%128), h, t, d] = values[t*128 + p, h, d]
    v3 = values.reshape(KT, 128, H, HD).transpose(1, 2, 0, 3)
    v16 = np.ascontiguousarray(v3).astype(NP_BF16)
    # wo16[p, h, o] = Wo[h*128+p, o]
    wo16 = np.ascontiguousarray(
        Wo.reshape(H, 128, D).transpose(1, 0, 2)).astype(NP_BF16)

    in_maps = []
    for c in range(N_CORES):
        in_maps.append({
            "xT": np.ascontiguousarray(xT8[:, :, c * SC:(c + 1) * SC]),
            "Wq": wq8, "kts": kts8, "v": v16, "Wo": wo16,
        })
    return in_maps


def kernel(x, Wq, keys, values, attn_scale, Wo):
    nc = _get_nc((), reps=1)
    in_maps = _make_in_maps(x, Wq, keys, values, attn_scale, Wo)
    res = run_bass_kernel_spmd(nc, in_maps, list(range(N_CORES)))
    out = np.concatenate([r["out"] for r in res.results], axis=0)
    return out.reshape(B, S, D).astype(np.float32)
